# revision 4
# baseline (speedup 1.0000x reference)
"""Causal squeeze-excite 1d on 8 TRN2 NeuronCores — fp16, software-pipelined.

v4 -> v5 (driven by the v4 trace: Sync prologue still serialized 5 DMAs
at ~2us fixed cost each -> x(0).b1 landed at 13.9us; epilogue ran its
muls on GPSIMD at 2.1us each, dragging the tail; ACT busy 42us was the
steady-state pacer):

1. Const DMAs move to the GPSIMD (SWDGE) queue so Sync carries only x
   loads; the first chunk's loads are split in half for an earlier mm1
   start.
2. relu(u + b1) leaves ACT: DVE tensor_scalar with a per-partition AP
   scalar computes (u + b1[p]) max 0 in one accelerated op. ACT now runs
   sigmoids only (~8.7us/chunk).
3. Epilogue gate-muls all on DVE (0.9us vs 2.1us on GPSIMD), stores
   per-cb on GPSIMD, interleaved behind each sigmoid.
"""

import numpy as np
from contextlib import ExitStack

import concourse.bass as bass
import concourse.bacc as bacc
import concourse.tile as tile
import concourse.mybir as mybir
from concourse.bass_utils import run_bass_kernel_spmd

F32 = mybir.dt.float32
F16 = mybir.dt.float16

N_CORES = 8
B, C, T = 16, 512, 4096
CSQ = 32          # squeeze dim
P = 128           # SBUF partitions


def build_nc(B_loc, cw, C_=C, T_=T, Tc=1024, TS=512):
    """Build the per-core Bass program. Shapes are compile-time constants."""
    d = 1.0 - 1.0 / cw
    NCB = C_ // P      # channel blocks (4)
    NTH = T_ // Tc     # time chunks (4)
    NTS = Tc // TS     # scan sub-blocks per chunk (2)
    BP = B_loc * CSQ   # packed scan partitions (64)
    W = NCB * Tc       # tile width (4096)
    WC16 = NCB * CSQ + C_   # fp16 const block cols: w1 (128) + w2 (512)

    nc = bacc.Bacc(trn_type="TRN2")
    x = nc.declare_dram_parameter("x", [B_loc, NTH, P, W], F16, isOutput=False)
    c16a = nc.declare_dram_parameter("c16a", [P, NCB * CSQ], F16,
                                     isOutput=False)
    c16b = nc.declare_dram_parameter("c16b", [P, C_], F16, isOutput=False)
    c32 = nc.declare_dram_parameter("c32", [P, NCB + 1], F32, isOutput=False)
    out = nc.declare_dram_parameter("out", [B_loc, NTH, P, W], F16,
                                    isOutput=True)

    with ExitStack() as ctx:
        tc = ctx.enter_context(tile.TileContext(nc))
        const = ctx.enter_context(tc.tile_pool(name="const", bufs=1))
        xpool = ctx.enter_context(tc.tile_pool(name="xp", bufs=8))
        opool = ctx.enter_context(tc.tile_pool(name="op", bufs=5))
        upool = ctx.enter_context(tc.tile_pool(name="up", bufs=3))
        hpool = ctx.enter_context(tc.tile_pool(name="hp", bufs=3))
        gpool = ctx.enter_context(tc.tile_pool(name="gp", bufs=17))
        cpool = ctx.enter_context(tc.tile_pool(name="cp", bufs=2))
        php = ctx.enter_context(tc.tile_pool(name="php", bufs=4, space="PSUM"))
        pgp = ctx.enter_context(tc.tile_pool(name="pgp", bufs=2, space="PSUM"))

        dconst = const.tile([BP, TS], F32, tag="dconst")
        nc.vector.memset(dconst[:], d)

        def load_x(th, b, split=1):
            xt = xpool.tile([P, W], F16, tag="x", name=f"x{b}")
            for s in range(split):
                w0, w1_ = s * W // split, (s + 1) * W // split
                nc.sync.dma_start(xt[:, w0:w1_], x[b, th, :, w0:w1_])
            return xt

        # Consts ride the Scalar HWDGE queue (warming up in parallel with
        # Sync, which carries only x loads, chunk 0's split in half).
        ct16a = const.tile([P, NCB * CSQ], F16, tag="c16a")
        nc.scalar.dma_start(ct16a[:], c16a[:])
        ct32 = const.tile([P, NCB + 1], F32, tag="c32")
        nc.scalar.dma_start(ct32[:], c32[:])
        xt0_b0 = load_x(0, 0, split=2)
        ct16b = const.tile([P, C_], F16, tag="c16b")
        nc.scalar.dma_start(ct16b[:], c16b[:])
        # b1 unsplit: each extra DMA on the warm-up-critical Sync queue
        # costs ~1.8us fixed, more than the overlap a split buys here.
        xt0_b1 = load_x(0, 1, split=1)
        w1_t = [ct16a[:, cb * CSQ:(cb + 1) * CSQ] for cb in range(NCB)]
        b2_t = ct32[:, 0:NCB]
        b1_t = ct32[0:BP, NCB:NCB + 1]

        def mm1(xts, ph_ts):
            # cb-outer so each w1 slice is loaded once per chunk; both
            # batches accumulate into disjoint partition halves (they run
            # concurrently on the PE as separate column groups).
            for cb in range(NCB):
                for ts in range(NTS):
                    for b in range(B_loc):
                        nc.tensor.matmul(
                            ph_ts[ts][b * CSQ:(b + 1) * CSQ, :], w1_t[cb],
                            xts[b][:, cb * Tc + ts * TS:cb * Tc + (ts + 1) * TS],
                            start=(cb == 0), stop=(cb == NCB - 1))

        state = {"carry": None}

        def scan_relu(th, ph_ts):
            ut = upool.tile([BP, NTS * TS], F16, tag="u")
            for ts in range(NTS):
                if th == 0 and ts == 0:
                    init = cpool.tile([BP, 1], F32, tag="c")
                    nc.vector.tensor_scalar_mul(init[:], ph_ts[0][:, 0:1],
                                                float(cw))
                    init_ap = init[:]
                else:
                    init_ap = state["carry"]
                nc.vector.tensor_tensor_scan(
                    ut[:, ts * TS:(ts + 1) * TS], dconst[:], ph_ts[ts][:],
                    init_ap, mybir.AluOpType.mult, mybir.AluOpType.add)
                state["carry"] = ut[:, (ts + 1) * TS - 1:(ts + 1) * TS]
            ht = hpool.tile([BP, NTS * TS], F16, tag="h")
            # relu(u + b1) on DVE: per-partition-scalar add, then max 0.
            nc.vector.tensor_scalar(
                ht[:], ut[:], b1_t, 0.0,
                mybir.AluOpType.add, mybir.AluOpType.max)
            return ht

        def mm2_sig(ht, gts, cbs):
            for cb in cbs:
                for b in range(B_loc):
                    pg = pgp.tile([P, Tc], F32, tag="pg")
                    for ts in range(NTS):
                        nc.tensor.matmul(
                            pg[:, ts * TS:(ts + 1) * TS],
                            ct16b[b * CSQ:(b + 1) * CSQ,
                                  cb * P:(cb + 1) * P],
                            ht[b * CSQ:(b + 1) * CSQ, ts * TS:(ts + 1) * TS],
                            start=True, stop=True)
                    nc.scalar.activation(
                        gts[b][cb][:], pg[:],
                        mybir.ActivationFunctionType.Sigmoid,
                        bias=b2_t[:, cb:cb + 1])

        def muls_store(th, xts, gts):
            mul_idx = 0
            for b in range(B_loc):
                ot = opool.tile([P, W], F16, tag="o", name=f"o{b}")
                for cb in range(NCB):
                    sl = slice(cb * Tc, (cb + 1) * Tc)
                    mul_eng = nc.vector if mul_idx % 4 < 3 else nc.gpsimd
                    mul_idx += 1
                    mul_eng.tensor_mul(ot[:, sl], xts[b][:, sl], gts[b][cb][:])
                nc.gpsimd.dma_start(out[b, th], ot[:])

        # ---- software-pipelined main loop ----
        xts_cur = [xt0_b0, xt0_b1]
        ph_cur = [php.tile([BP, TS], F32, tag="ph", name=f"ph{i}")
                  for i in range(NTS)]
        mm1(xts_cur, ph_cur)
        prev = None  # (th, xts, gts) waiting for gate-mul + store
        for th in range(NTH):
            gts = [[gpool.tile([P, Tc], F16, tag="g", name=f"g{b}_{cb}")
                    for cb in range(NCB)] for b in range(B_loc)]
            ht = scan_relu(th, ph_cur)
            # muls for th-1 now: every sigmoid(th-1) precedes relu(th) in
            # ACT order, so these never stall DVE, and they free x/g tiles
            # at the earliest possible point.
            if prev is not None:
                muls_store(*prev)
            if th + 1 < NTH:
                # mm2 for cb0 first so ACT has sigmoids to chew on while
                # the PE runs mm1 for the next chunk.
                mm2_sig(ht, gts, [0])
                xts_next = [load_x(th + 1, b) for b in range(B_loc)]
                ph_next = [php.tile([BP, TS], F32, tag="ph", name=f"phn{i}")
                           for i in range(NTS)]
                mm1(xts_next, ph_next)
                mm2_sig(ht, gts, range(1, NCB))
                prev = (th, xts_cur, gts)
                xts_cur, ph_cur = xts_next, ph_next
            else:
                # Epilogue: interleave mul + per-cb 256KB store right
                # behind each sigmoid to shrink the serial tail.
                ots = [opool.tile([P, W], F16, tag="o", name=f"oe{b}")
                       for b in range(B_loc)]
                for cb in range(NCB):
                    mm2_sig(ht, gts, [cb])
                    for b in range(B_loc):
                        sl = slice(cb * Tc, (cb + 1) * Tc)
                        nc.vector.tensor_mul(ots[b][:, sl], xts_cur[b][:, sl],
                                             gts[b][cb][:])
                        nc.gpsimd.dma_start(out[b, th, :, sl],
                                            ots[b][:, sl])
    nc.compile()
    return nc


def make_in_maps(x, w1, b1, w2, b2, cw, n_cores=N_CORES):
    """Host-side shard + pre-tile + const pack. Returns per-core inputs."""
    a = 1.0 / cw
    Tc = 1024
    ncb = w2.shape[0] // P
    csq = w1.shape[0]
    # fp16 const blocks: w1 (a-folded, [p, cb*csq+s]) and w2 (transposed,
    # batch-duplicated rows, zero-padded to 128 partitions).
    w1p = (np.asarray(w1) * a).T.reshape(ncb, P, csq).transpose(1, 0, 2) \
        .reshape(P, ncb * csq)
    c16a = np.ascontiguousarray(w1p.astype(np.float16))
    w2T = np.tile(np.asarray(w2).T, (2, 1))        # [64, C]
    w2pad = np.zeros((P, w2T.shape[1]), np.float32)
    w2pad[:w2T.shape[0]] = w2T
    c16b = np.ascontiguousarray(w2pad.astype(np.float16))
    # fp32 const block [P, ncb+1]: b2 ([p, cb]) | b1 (batch-dup, padded).
    b2c = np.asarray(b2).reshape(ncb, P).T
    b1c = np.zeros((P, 1), np.float32)
    b1c[:2 * csq, 0] = np.tile(np.asarray(b1), 2)
    c32 = np.ascontiguousarray(
        np.concatenate([b2c, b1c], axis=1).astype(np.float32))
    b_loc = x.shape[0] // n_cores
    xh = np.asarray(x).astype(np.float16)
    Bt, Ct, Tt = xh.shape
    nth = Tt // Tc
    xp = np.ascontiguousarray(
        xh.reshape(Bt, ncb, P, nth, Tc).transpose(0, 3, 2, 1, 4)
        .reshape(Bt, nth, P, ncb * Tc))
    return [
        {"x": xp[i * b_loc:(i + 1) * b_loc], "c16a": c16a, "c16b": c16b,
         "c32": c32}
        for i in range(n_cores)
    ]


def untile_out(res_list, n_cores=N_CORES):
    """[B_loc, NTH, P, NCB*Tc] fp16 per core -> [B, C, T] fp32."""
    o = np.concatenate([r["out"] for r in res_list], axis=0)
    Bt, nth, Pt, Wt = o.shape
    ncb = Wt // 1024
    full = o.reshape(Bt, nth, Pt, ncb, 1024).transpose(0, 3, 2, 1, 4)
    return np.ascontiguousarray(
        full.reshape(Bt, ncb * Pt, nth * 1024)).astype(np.float32)


_NC_CACHE = {}


def kernel(x, w1, b1, w2, b2, context_window):
    cw = int(context_window)
    x = np.asarray(x)
    key = (cw, x.shape)
    if key not in _NC_CACHE:
        _NC_CACHE[key] = build_nc(x.shape[0] // N_CORES, cw)
    nc = _NC_CACHE[key]
    in_maps = make_in_maps(x, w1, b1, w2, b2, cw)
    res = run_bass_kernel_spmd(nc, in_maps, core_ids=list(range(N_CORES)))
    return untile_out(res.results)
